# revision 37
# baseline (speedup 1.0000x reference)
"""Trainium2 Bass kernel for nn_CGLayer (gnn_message_passing) — fused single launch.

Math (reference semantics):
  sx[b,n,g]      = sum_j s_l[b,n,j,:]                 g = (l2,m2) in [0,9)
  q[b,n,p,c]     = sx[b,n,g(p)] * v9[b,n,v(p),c]      p over 80 used products
  h[b,i,p,c]     = sum_m conn[b,i,m] * q[b,m,p,c]     (TensorE; conn and q exact
                                                       /near-exact in fp16)
  mp[b,i,row,c]  = sum_{p in row} CG[row,p] * h[b,i,p,c]   (51 rows, fp32)
  out            = mp * 64 / ((2L+1)*||mp_L||_F)      per degree L (host, 3 scalars)

Sharding: 8 cores = (batch b, half h). Core (b,h) reduces s_l[b, :, j-half h]
(18 MiB); two pipelined pairwise AllReduces (m-tiles 0-3, then 4-7) complete
the j-sum; the core then computes mp rows for receiver half h. The CG combine
runs AFTER the message-passing matmul (512 receiver rows, 4-chunk fused).
Products are processed in column sweeps [16,32,32] so every matmul is a full
512-wide PSUM bank; matmuls run sender-tile-outer so weights amortize and
PSUM accumulation starts as soon as each sender tile's products exist.
"""

import numpy as np
from math import factorial

import ml_dtypes

from concourse import bacc, tile, mybir
from concourse.bass_utils import run_bass_kernel_spmd

B, N, C = 4, 1024, 64
HALF = N // 2
NT = N // 128          # m-tiles per batch
NIC = HALF // 128      # receiver chunks per core
NCORES = 8
LOFF = [0, 1, 4]

AluOp = mybir.AluOpType
dt = mybir.dt
QDT = dt.float16       # product dtype for the TensorE pass


# ---------------------------------------------------------------- CG tables
def _cg_coeff(l1, m1, l2, m2, L, M):
    if m1 + m2 != M or not (abs(l1 - l2) <= L <= l1 + l2):
        return 0.0
    f = factorial
    pre = ((2 * L + 1) * f(L + l1 - l2) * f(L - l1 + l2) * f(l1 + l2 - L)
           / f(l1 + l2 + L + 1)) ** 0.5
    pre *= (f(L + M) * f(L - M) * f(l1 - m1) * f(l1 + m1) * f(l2 - m2)
            * f(l2 + m2)) ** 0.5
    s = 0.0
    for k in range(0, l1 + l2 - L + 1):
        dens = [k, l1 + l2 - L - k, l1 - m1 - k, l2 + m2 - k,
                L - l2 + m1 + k, L - l1 - m2 + k]
        if any(d < 0 for d in dens):
            continue
        term = (-1.0) ** k
        for d in dens:
            term /= f(d)
        s += term
    return pre * s


def _build_tables():
    rows = []
    for L in range(3):
        frags = [(l1, l2) for l1 in range(3) for l2 in range(3)
                 if abs(l1 - l2) <= L <= l1 + l2]
        for k in range(2 * L + 1):
            for (l1, l2) in frags:
                rows.append((L, k, l1, l2))
    entries = []           # per row: list of (v9col, sxcol, coeff)
    for (L, k, l1, l2) in rows:
        M = k - L
        es = []
        for i in range(2 * l1 + 1):
            m1 = i - l1
            m2 = M - m1
            if abs(m2) <= l2:
                c = _cg_coeff(l1, m1, l2, m2, L, M)
                if c != 0.0:
                    es.append((LOFF[l1] + i, LOFF[l2] + l2 + m2, c))
        entries.append(es)
    return rows, entries


ROWS, ENTRIES = _build_tables()
NROWS = len(ROWS)                                   # 51
NCH = NROWS * C                                     # 3264
_L_NROWS = [sum(1 for r in ROWS if r[0] == L) for L in range(3)]
L_RANGES = []
_c0 = 0
for L in range(3):
    L_RANGES.append((_c0, _c0 + _L_NROWS[L] * C))
    _c0 += _L_NROWS[L] * C

# product columns: (sxcol g)-major x (vcol), dropping unused (8,8) -> 80 cols
GROUP_NV = [9] * 8 + [8]
NP_ = sum(GROUP_NV)                                 # 80
P_START = np.cumsum([0] + GROUP_NV).tolist()
PIDX = {(g, v): P_START[g] + v
        for g in range(9) for v in range(GROUP_NV[g])}

ROW_PENTRIES = []
for es in ENTRIES:
    pes = sorted((PIDX[(sxcol, vcol)], coeff) for (vcol, sxcol, coeff) in es)
    ROW_PENTRIES.append(pes)

# column-range sweeps; every sweep width is a multiple of 8 (full PSUM banks)
SWEEP_RANGES = [(0, 16), (16, 48), (48, 80)]
MAXW = max(b - a for (a, b) in SWEEP_RANGES)


def _sweep_builds(c0, c1):
    """group-write segments intersecting [c0, c1): (g, va, vb, local_off)."""
    segs = []
    for g in range(9):
        ga, gb = P_START[g], P_START[g] + GROUP_NV[g]
        a, b = max(ga, c0), min(gb, c1)
        if a < b:
            segs.append((g, a - ga, b - ga, a - c0))
    return segs


SWEEPS = []
for (c0, c1) in SWEEP_RANGES:
    w = c1 - c0
    chunks = [(off, 8) for off in range(0, w, 8)]
    entries = []
    for r, pes in enumerate(ROW_PENTRIES):
        for k, (p, cf) in enumerate(pes):
            if c0 <= p < c1:
                entries.append((r, p - c0, cf, k == 0))
    SWEEPS.append((c0, w, chunks, _sweep_builds(c0, c1), entries))


# ---------------------------------------------------------------- program
def build_fused():
    nc = bacc.Bacc("TRN2", target_bir_lowering=False, debug=False,
                   num_devices=NCORES)
    s_in = [nc.dram_tensor(f"s{l}h", [N, HALF, 2 * l + 1], dt.float32,
                           kind="ExternalInput") for l in range(3)]
    v9_in = nc.dram_tensor("v9", [N, 9, C], dt.float32, kind="ExternalInput")
    adjT_in = nc.dram_tensor("adjT", [N, HALF], QDT, kind="ExternalInput")
    mp_out = nc.dram_tensor("mp", [HALF, NCH], dt.float32, kind="ExternalOutput")
    NQ = 2                     # AllReduce groups (4 m-tiles each)
    NTQ = NT // NQ
    ar_in = [nc.dram_tensor(f"ar_in{i}", [NTQ * 128, 9], dt.float32)
             for i in range(NQ)]
    ar_out = [nc.dram_tensor(f"ar_out{i}", [NTQ * 128, 9], dt.float32)
              for i in range(NQ)]
    groups = [[0, 1], [2, 3], [4, 5], [6, 7]]

    with tile.TileContext(nc) as tc:
        with (tc.tile_pool(name="const", bufs=1) as cpool,
              tc.tile_pool(name="stream", bufs=2) as spool,
              tc.tile_pool(name="hi", bufs=2) as hpool,
              tc.tile_pool(name="h4", bufs=1) as h4pool,
              tc.tile_pool(name="psum", bufs=1, space="PSUM") as pspool):
            adjT_sb = cpool.tile([128, NT, HALF], QDT)
            sxp_q = [cpool.tile([128, NTQ, 9], dt.float32, name=f"sxp{i}")
                     for i in range(NQ)]
            sx_q = [cpool.tile([128, NTQ, 9], dt.float32, name=f"sx{i}")
                    for i in range(NQ)]
            out_p = [cpool.tile([128, 2, NROWS, C], dt.float32, name=f"out{i}")
                     for i in range(2)]
            # prefetch v9 for the first sweep's early tiles ahead of the big
            # s-tensor stream
            v9pre = [cpool.tile([128, 9, C], dt.float32, name=f"v9p{t}")
                     for t in range(4)]
            for t in range(4):
                nc.gpsimd.dma_start(v9pre[t][:, :, :],
                                    v9_in[t * 128:(t + 1) * 128, :, :])
            for t in range(NT):
                nc.sync.dma_start(adjT_sb[:, t, :],
                                  adjT_in[t * 128:(t + 1) * 128, :])

            # ---- phase A: j-half reduction of s (DVE: s1,s2; GpSimd: s0)
            def reduce_dve(st, t, l, d):
                nc.vector.tensor_reduce(
                    sxp_q[t // NTQ][:, t % NTQ, LOFF[l]:LOFF[l] + d],
                    st[:, :, :].transpose([0, 2, 1]),
                    axis=mybir.AxisListType.X, op=AluOp.add)

            def reduce_gp(st, t, l, d):
                n = HALF // 2
                while n >= 1:
                    nc.gpsimd.tensor_add(
                        st[:, 0:n, :], st[:, 0:n, :], st[:, n:2 * n, :])
                    n //= 2
                nc.gpsimd.tensor_copy(
                    sxp_q[t // NTQ][:, t % NTQ, LOFF[l]:LOFF[l] + d],
                    st[:, 0, :])

            # split each s-tile load into parallel j-chunks: with only ~6
            # whole-tile DMAs in flight the per-queue bandwidth (~20-50 GB/s)
            # caps the stream far below the HBM peak
            SPLITS = {0: 1, 1: 2, 2: 4}
            for t in range(NT):
                sl = slice(t * 128, (t + 1) * 128)
                for l in range(3):
                    d = 2 * l + 1
                    st = spool.tile([128, HALF, d], dt.float32, tag=f"s{l}")
                    jc = HALF // SPLITS[l]
                    for c2 in range(SPLITS[l]):
                        jsl = slice(c2 * jc, (c2 + 1) * jc)
                        nc.sync.dma_start(st[:, jsl, :], s_in[l][sl, jsl, :])
                    if l == 0 or (l == 1 and t % 2 == 1):
                        reduce_gp(st, t, l, d)
                    else:
                        reduce_dve(st, t, l, d)
                # ---- phase B: pipelined pairwise AllReduce per m-tile group.
                # Engine choice avoids FIFO head-of-line stalls: the ar_in
                # trigger rides DVE (waits on reduces DVE mostly produced
                # itself), the post-collective sx trigger rides ScalarE
                # (whose next work needs sx anyway); both get semaphores
                # independent of the big s-stream.
                if t % NTQ == NTQ - 1:
                    qf = t // NTQ
                    nc.gpsimd.dma_start(
                        ar_in[qf].rearrange("(t p) c -> p t c", p=128),
                        sxp_q[qf][:, :, :])
                    nc.gpsimd.collective_compute(
                        "AllReduce", AluOp.add, replica_groups=groups,
                        ins=[ar_in[qf][:]], outs=[ar_out[qf][:]])

            # ---- phases C/D/E per sweep; product builds are emitted one
            # sweep ahead of the matmul stream so ScalarE pre-builds the next
            # sweep's products instead of FIFO-stalling on PSUM copies
            def emit_his(si):
                c0, w, chunks, builds, entries = SWEEPS[si]
                his = []
                for t in range(NT):
                    # post-collective sx fetch rides ScalarE, emitted just
                    # before the first product build that needs it
                    if si == 0 and t % NTQ == 0:
                        qf = t // NTQ
                        nc.scalar.dma_start(
                            sx_q[qf][:, :, :],
                            ar_out[qf].rearrange("(t p) c -> p t c", p=128))
                    if si == 0 and t < 4:
                        v9t = v9pre[t]
                    else:
                        v9t = spool.tile([128, 9, C], dt.float32, tag="v9",
                                         name=f"v9_{si}_{t}")
                        nc.gpsimd.dma_start(v9t[:, :, :],
                                            v9_in[t * 128:(t + 1) * 128, :, :])
                    hi = hpool.tile([128, MAXW, C], QDT, tag=f"hi{t}",
                                    name=f"hi_{si}_{t}")
                    sxt = sx_q[t // NTQ]
                    for bi, (g, va, vb, loff) in enumerate(builds):
                        if bi % 3 == 2:     # 1/3 of product builds on DVE
                            nc.vector.tensor_scalar_mul(
                                hi[:, loff:loff + (vb - va), :],
                                v9t[:, va:vb, :], sxt[:, t % NTQ, g:g + 1])
                        else:               # 2/3 on ScalarE
                            nc.scalar.activation(
                                hi[:, loff:loff + (vb - va), :],
                                v9t[:, va:vb, :],
                                mybir.ActivationFunctionType.Copy,
                                scale=sxt[:, t % NTQ, g:g + 1])
                    his.append(hi)
                return his

            def emit_mm_e(si, his):
                c0, w, chunks, builds, entries = SWEEPS[si]
                # receiver-pair h tiles; sender-tile-outer matmuls in waves
                # so live PSUM banks never exceed 8
                h4p = [h4pool.tile([128, 2, MAXW, C], dt.float32,
                                   name=f"h4_{si}_{pr}", tag=f"h4{pr}")
                       for pr in range(2)]
                nwave = max(1, (len(chunks) * NIC) // 8)
                icw = NIC // nwave
                for wv in range(nwave):
                    wave_ics = range(wv * icw, (wv + 1) * icw)
                    pss = {
                        (ic, ci): pspool.tile(
                            [128, cw * C], dt.float32,
                            name=f"ps_{si}_{ic}_{ci}",
                            tag=f"ps{(ic % icw) * len(chunks) + ci}")
                        for ic in wave_ics
                        for ci, (coff, cw) in enumerate(chunks)}
                    for t in range(NT):
                        for ic in wave_ics:
                            for ci, (coff, cw) in enumerate(chunks):
                                nc.tensor.matmul(
                                    pss[(ic, ci)][:, :],
                                    adjT_sb[:, t, ic * 128:(ic + 1) * 128],
                                    his[t][:, coff:coff + cw, :],
                                    start=(t == 0), stop=(t == NT - 1))
                    for ic in wave_ics:
                        for ci, (coff, cw) in enumerate(chunks):
                            dst = (h4p[ic // 2][:, ic % 2, coff:coff + cw, :]
                                   .rearrange("p a b -> p (a b)"))
                            # alternate engines so the copy burst at each wave
                            # boundary doesn't serialize on ScalarE
                            if (ic * len(chunks) + ci) % 2:
                                nc.vector.tensor_copy(dst, pss[(ic, ci)][:, :])
                            else:
                                nc.scalar.copy(dst, pss[(ic, ci)][:, :])

                # ---- phase E: CG combine per receiver pair
                for pr in range(2):
                    for (r, lp, cf, is_init) in entries:
                        if is_init:
                            nc.vector.tensor_scalar_mul(
                                out_p[pr][:, :, r, :], h4p[pr][:, :, lp, :],
                                float(cf))
                        else:
                            nc.vector.scalar_tensor_tensor(
                                out_p[pr][:, :, r, :], h4p[pr][:, :, lp, :],
                                float(cf), out_p[pr][:, :, r, :],
                                op0=AluOp.mult, op1=AluOp.add)

            prev = emit_his(0)
            for si in range(len(SWEEPS)):
                nxt = emit_his(si + 1) if si + 1 < len(SWEEPS) else None
                emit_mm_e(si, prev)
                prev = nxt

            for ic in range(NIC):
                nc.sync.dma_start(
                    mp_out[ic * 128:(ic + 1) * 128, :],
                    out_p[ic // 2][:, ic % 2, :, :].rearrange(
                        "p a b -> p (a b)"))
    nc.compile()
    return nc


_programs = {}


def _get_program():
    if "fused" not in _programs:
        _programs["fused"] = build_fused()
    return _programs["fused"]


# ---------------------------------------------------------------- host driver
def kernel(v0, v1, v2, s0, s1, s2, conn, _trace=False, _results=None):
    v0 = np.asarray(v0, np.float32)
    v1 = np.asarray(v1, np.float32)
    v2 = np.asarray(v2, np.float32)
    conn = np.asarray(conn)
    s = [np.asarray(x, np.float32) for x in (s0, s1, s2)]

    v9 = np.concatenate([v0, v1, v2], axis=2)                  # [B, N, 9, C]
    adjT = conn.transpose(0, 2, 1).astype(np.float16)          # [B, m, i]

    core_ids = list(range(NCORES))
    in_maps = []
    for k in core_ids:
        b, h = divmod(k, 2)
        jsl = slice(h * HALF, (h + 1) * HALF)
        m = {f"s{l}h": np.ascontiguousarray(s[l][b, :, jsl, :, 0])
             for l in range(3)}
        m["v9"] = v9[b]
        m["adjT"] = np.ascontiguousarray(adjT[b, :, h * HALF:(h + 1) * HALF])
        in_maps.append(m)

    r = run_bass_kernel_spmd(_get_program(), in_maps, core_ids, trace=_trace)
    mp = np.empty((B, N, NCH), np.float32)
    for k in core_ids:
        b, h = divmod(k, 2)
        mp[b, h * HALF:(h + 1) * HALF] = r.results[k]["mp"]

    if _results is not None:
        _results.append(r)

    out = np.empty_like(mp)
    for L, (c0, c1) in enumerate(L_RANGES):
        seg = mp[:, :, c0:c1]
        nf = (2 * L + 1) * np.linalg.norm(seg.astype(np.float64))
        out[:, :, c0:c1] = (seg.astype(np.float64) / (nf / C)).astype(np.float32)
    return out
